# revision 21
# baseline (speedup 1.0000x reference)
"""Trainium2 Bass kernel for nn_EnhancedGNNModel (3-layer EdgeConv GNN + BN + pooling).

Strategy (8-core SPMD, graph partitioned by destination node):
 - Nodes are range-sharded across cores (NPC = N/8 each). Edges are sorted by
   dst and partitioned to the core owning dst; within a core they are grouped
   by 128-node destination blocks and padded to 128-edge tiles.
 - Algebraic restructuring: the edge MLP's first matmul splits into node-level
   matmuls (A' = h@eW1[:H] + eb1 at dst, B = h@eW1[H:2H] at src) plus a rank-1
   term ea*eW1[2H]; the second edge matmul (@eW2) commutes with segment-sum and
   is applied after aggregation at node level.  Per-edge work is therefore just
   gather(A'[dst]) + gather(B[src]) + ea*w, relu, and a selector-matrix matmul
   that performs the segment-sum into PSUM.
 - All node tensors are kept feature-major [128 feat, nodes] so every node
   matmul uses the weight as stationary lhsT with no transposes.
 - Per layer, only the B table needs cross-core data: one AllGather of the
   node-sharded B (25.6MB full size).  BN statistics use a [128,2] AllReduce;
   the final pooled sums use a [128,G] AllReduce.
"""
import os
import numpy as np
from contextlib import ExitStack

from concourse import bacc, bass, mybir
import concourse.tile as tile
from concourse import bass2jax

P = 128
f32 = mybir.dt.float32
i16 = mybir.dt.int16
i32 = mybir.dt.int32

# real problem sizes (hardcoded; test harnesses may build other sizes)
N_NODES = 50000
N_EDGES = 800000
D_H = 128
N_GRAPHS = 8
CORES = 8
BN_EPS = 1e-5
NODE_CHUNK = 512


# --------------------------------------------------------------------------
# host-side graph preprocessing (pure function of edge_index / batch)
# --------------------------------------------------------------------------

def preprocess(edge_index, batch, edge_attr, n_nodes, n_graphs, cores):
    """Partition edges by dst core/block, pad to 128-edge tiles, build gather
    indices, selector slots, degrees and pooling selectors.  Returns per-core
    arrays (identical shapes across cores) plus the shared block structure."""
    src = np.asarray(edge_index[0], dtype=np.int64)
    dst = np.asarray(edge_index[1], dtype=np.int64)
    ea = np.asarray(edge_attr, dtype=np.float32).reshape(-1)
    batch = np.asarray(batch, dtype=np.int64)
    E = src.shape[0]
    npc = n_nodes // cores
    nblk = (npc + P - 1) // P
    half = n_nodes // 2

    order = np.argsort(dst, kind="stable")
    s_src, s_dst = src[order], dst[order]
    s_ea = ea[order]

    deg = np.bincount(dst, minlength=n_nodes).astype(np.float32)

    # per (core, block, stream): edge index lists
    # stream 0: src < half (gather from B_full[:half]); stream 1: src >= half
    counts = np.zeros((cores, nblk, 2), dtype=np.int64)
    # boundaries of each core's edge range in the sorted order
    core_starts = np.searchsorted(s_dst, np.arange(cores + 1) * npc)
    block_edge_lists = [[None] * nblk for _ in range(cores)]
    for c in range(cores):
        e0, e1 = core_starts[c], core_starts[c + 1]
        d_loc = s_dst[e0:e1] - c * npc
        blk = d_loc // P
        blk_starts = np.searchsorted(blk, np.arange(nblk + 1))
        for b in range(nblk):
            i0, i1 = e0 + blk_starts[b], e0 + blk_starts[b + 1]
            lo_mask = s_src[i0:i1] < half
            idx_lo = np.nonzero(lo_mask)[0] + i0
            idx_hi = np.nonzero(~lo_mask)[0] + i0
            block_edge_lists[c][b] = (idx_lo, idx_hi)
            counts[c, b, 0] = len(idx_lo)
            counts[c, b, 1] = len(idx_hi)

    # common (max-over-cores) tile structure
    t_lo = np.maximum((counts[:, :, 0].max(axis=0) + P - 1) // P, 1)
    t_hi = np.maximum((counts[:, :, 1].max(axis=0) + P - 1) // P, 1)
    t_blk = t_lo + t_hi
    T = int(t_blk.sum())

    per_core = []
    for c in range(cores):
        idxA = np.zeros((T * P,), dtype=np.int64)    # dst_local in [0, npc)
        idxB = np.zeros((T * P,), dtype=np.int64)    # src (rebased per stream)
        slot = np.full((T * P,), -1.0, dtype=np.float32)
        eacol = np.zeros((T * P,), dtype=np.float32)
        t0 = 0
        for b in range(nblk):
            idx_lo, idx_hi = block_edge_lists[c][b]
            for s_i, (eids, tcnt) in enumerate(((idx_lo, t_lo[b]), (idx_hi, t_hi[b]))):
                base = t0 * P
                n = len(eids)
                idxA[base:base + n] = s_dst[eids] - c * npc
                bsrc = s_src[eids] - (half if s_i == 1 else 0)
                idxB[base:base + n] = bsrc
                slot[base:base + n] = (s_dst[eids] - c * npc - b * P).astype(np.float32)
                eacol[base:base + n] = s_ea[eids]
                t0 += int(tcnt)
        assert t0 == T

        # [128, T] tile-column layout: slot i -> (partition i%128, tile i//128)
        def tiled(a, dtype):
            return a.reshape(T, P).T.astype(dtype).copy()

        # wrapped int16 index layout for dma_gather: idx i at [i%16, i//16],
        # replicated over the 8 16-partition groups
        def wrap(a):
            w = a.reshape(-1, 16).T.astype(np.int16)  # [16, T*8]
            return np.tile(w, (8, 1)).copy()

        deg_c = deg[c * npc:(c + 1) * npc]
        deg_row = np.zeros((1, nblk * P), dtype=np.float32)
        deg_row[0, :npc] = deg_c

        pool_sel = np.zeros((P, nblk * n_graphs), dtype=np.float32)
        bat_c = batch[c * npc:(c + 1) * npc]
        for b in range(nblk):
            nb = min(P, npc - b * P)
            for s_i in range(nb):
                g = bat_c[b * P + s_i]
                pool_sel[s_i, b * n_graphs + g] = 1.0

        per_core.append(dict(
            idxA=wrap(idxA), idxB=wrap(idxB),
            slot=tiled(slot, np.float32), eacol=tiled(eacol, np.float32),
            deg_row=deg_row, pool_sel=pool_sel,
        ))

    cnts = np.bincount(batch, minlength=n_graphs).astype(np.float32)
    inv_cnt = (1.0 / np.maximum(cnts, 1.0)).astype(np.float32)

    struct = dict(t_lo=t_lo.astype(int), t_hi=t_hi.astype(int), T=T,
                  nblk=nblk, npc=npc, half=half)
    return per_core, struct, inv_cnt


# --------------------------------------------------------------------------
# device program
# --------------------------------------------------------------------------

def build_program(n_nodes, n_graphs, cores, struct):
    dbg = set(os.environ.get("GNN_DEBUG", "").split(","))
    nblk, npc, T = struct["nblk"], struct["npc"], struct["T"]
    t_lo, t_hi = struct["t_lo"], struct["t_hi"]
    half = struct["half"]
    ACT = mybir.ActivationFunctionType
    ALU = mybir.AluOpType

    nc = bacc.Bacc(None)

    def din(name, shape, dt=f32):
        return nc.dram_tensor(name, shape, dt, kind="ExternalInput")

    xT_in = din("xT", [P, npc])
    # per-layer stacked weights
    nW1_in = din("nW1", [3 * P, P])
    nb1_in = din("nb1", [P, 3])
    nW2_in = din("nW2", [3 * P, P])
    nb2_in = din("nb2", [P, 3])
    eW1d_in = din("eW1d", [3 * P, P])
    eW1s_in = din("eW1s", [3 * P, P])
    eb1_in = din("eb1", [P, 3])
    wbc_in = din("wbc", [3 * P, P])       # eW1[2H] row replicated to 128 partitions
    eW2_in = din("eW2", [3 * P, P])
    eb2row_in = din("eb2row", [1, 3 * P])
    bng_in = din("bng", [P, 3])
    bnb_in = din("bnb", [P, 3])
    fc1W_in = din("fc1W", [P, P])
    fc1b_in = din("fc1b", [8, P])         # replicated across 8 partitions
    fc2W_in = din("fc2W", [P, 1])
    fc2b_in = din("fc2b", [8, 1])

    idxA_in = din("idxA", [P, T * 8], i16)
    idxB_in = din("idxB", [P, T * 8], i16)
    slot_in = din("slot", [P, T])
    ea_in = din("ea", [P, T])
    degrow_in = din("deg_row", [1, nblk * P])
    poolsel_in = din("pool_sel", [P, nblk * n_graphs])
    invcnt_in = din("inv_cnt", [P, n_graphs])  # replicated across partitions
    ident_in = din("ident", [P, P])

    out_o = nc.dram_tensor("out", [n_graphs, 1], f32, kind="ExternalOutput")

    n_chunks = (npc + NODE_CHUNK - 1) // NODE_CHUNK

    def chunks():
        for ch in range(n_chunks):
            j0 = ch * NODE_CHUNK
            yield j0, min(NODE_CHUNK, npc - j0)

    with ExitStack() as ctx:
        tc = ctx.enter_context(tile.TileContext(nc))
        sb = ctx.enter_context(tc.tile_pool(name="sb", bufs=1))
        sb2 = ctx.enter_context(tc.tile_pool(name="sb2", bufs=2))
        sb3 = ctx.enter_context(tc.tile_pool(name="sb3", bufs=3))
        psn = ctx.enter_context(tc.tile_pool(name="psn", bufs=2, space="PSUM"))
        pss = ctx.enter_context(tc.tile_pool(name="pss", bufs=2, space="PSUM"))
        psm = ctx.enter_context(tc.tile_pool(name="psm", bufs=2, space="PSUM"))
        psp = ctx.enter_context(tc.tile_pool(name="psp", bufs=1, space="PSUM"))
        dram = ctx.enter_context(tc.tile_pool(name="dram", bufs=1, space="DRAM"))

        # ---------- resident loads ----------
        def load(ap_in, shape, dt=f32, pool=sb):
            t = pool.tile(shape, dt, tag=f"ld_{ap_in.name}")
            nc.sync.dma_start(out=t[:], in_=ap_in[:])
            return t

        xT = sb.tile([P, npc], f32)
        nc.sync.dma_start(out=xT[:], in_=xT_in[:])

        def load3(ap_in, name):
            ts = []
            for l in range(3):
                t = sb.tile([P, P], f32, tag=f"{name}{l}")
                nc.sync.dma_start(out=t[:], in_=ap_in[l * P:(l + 1) * P, :])
                ts.append(t)
            return ts

        nW1 = load3(nW1_in, "nW1")
        nb1 = load(nb1_in, [P, 3])
        nW2 = load3(nW2_in, "nW2")
        nb2 = load(nb2_in, [P, 3])
        eW1d = load3(eW1d_in, "eW1d")
        eW1s = load3(eW1s_in, "eW1s")
        eb1 = load(eb1_in, [P, 3])
        wbc = load3(wbc_in, "wbc")
        eW2 = load3(eW2_in, "eW2")
        eb2row = load(eb2row_in, [1, 3 * P])
        bng = load(bng_in, [P, 3])
        bnb = load(bnb_in, [P, 3])
        fc1W = load(fc1W_in, [P, P])
        fc1b = load(fc1b_in, [8, P])
        fc2W = load(fc2W_in, [P, 1])
        fc2b = load(fc2b_in, [8, 1])
        idxA = load(idxA_in, [P, T * 8], i16)
        idxB = load(idxB_in, [P, T * 8], i16)
        slot = load(slot_in, [P, T])
        eac = load(ea_in, [P, T])
        degrow = load(degrow_in, [1, nblk * P])
        poolsel = load(poolsel_in, [P, nblk * n_graphs])
        invcnt = load(invcnt_in, [P, n_graphs])
        ident = load(ident_in, [P, P])

        iota_i = sb.tile([P, P], i32)
        iota_f = sb.tile([P, P], f32)
        nc.gpsimd.iota(out=iota_i[:], pattern=[[1, P]], base=0, channel_multiplier=0)
        nc.vector.tensor_copy(out=iota_f[:], in_=iota_i[:])

        # DRAM gather tables (per layer to avoid WAR on gather sources)
        Ap_nm_l = []
        Bsh_nm_l = []
        for l in range(3):
            ap_t = dram.tile([npc, P], f32, tag=f"Ap{l}", name=f"Ap_nm{l}")
            bs_t = dram.tile([npc, P], f32, tag=f"Bs{l}", name=f"Bsh_nm{l}")
            Ap_nm_l.append(ap_t)
            Bsh_nm_l.append(bs_t)
        Bfull_l = []
        stats_out_l = []
        for l in range(3):
            aspace = "Local" if "nocoll" in dbg else "Shared"
            bf = dram.tile([n_nodes, P], f32, addr_space=aspace, tag=f"Bfull{l}",
                           name=f"Bfull{l}")
            so = dram.tile([P, 2], f32, addr_space="Shared", tag=f"stats_out{l}",
                           name=f"stats_out{l}")
            Bfull_l.append(bf)
            stats_out_l.append(so)
        stats_in = dram.tile([P, 2], f32)
        pool_in = dram.tile([P, n_graphs], f32)
        pool_out = dram.tile([P, n_graphs], f32, addr_space="Shared")

        h2T = sb.tile([P, npc], f32)
        xn0 = sb.tile([P, npc], f32, tag="xnext0")
        xn1 = sb.tile([P, npc], f32, tag="xnext1")
        xn = [xn0, xn1]

        x_cur = xT
        for l in range(3):
            Bfull = Bfull_l[l]
            stats_out = stats_out_l[l]
            Ap_nm = Ap_nm_l[l]
            Bsh_nm = Bsh_nm_l[l]
            # ---------- node phase (feature-major) ----------
            for j0, w in chunks():
                jsl = slice(j0, j0 + w)
                h1p = psn.tile([P, NODE_CHUNK], f32, tag="psn", space="PSUM")
                nc.tensor.matmul(out=h1p[:, :w], lhsT=nW1[l][:],
                                 rhs=x_cur[:, jsl], start=True, stop=True)
                h1 = sb2.tile([P, NODE_CHUNK], f32, tag="h1", bufs=1)
                nc.scalar.activation(out=h1[:, :w], in_=h1p[:, :w],
                                     func=ACT.Relu, bias=nb1[:, l:l + 1])
                h2p = psn.tile([P, NODE_CHUNK], f32, tag="psn", space="PSUM")
                nc.tensor.matmul(out=h2p[:, :w], lhsT=nW2[l][:],
                                 rhs=h1[:, :w], start=True, stop=True)
                nc.vector.tensor_scalar(out=h2T[:, jsl], in0=h2p[:, :w],
                                        scalar1=nb2[:, l:l + 1], scalar2=None,
                                        op0=ALU.add)
                # A' = h@eW1d + eb1 ; B = h@eW1s   (feature-major chunks)
                app = psn.tile([P, NODE_CHUNK], f32, tag="psn", space="PSUM")
                nc.tensor.matmul(out=app[:, :w], lhsT=eW1d[l][:],
                                 rhs=h2T[:, jsl], start=True, stop=True)
                apch = sb2.tile([P, NODE_CHUNK], f32, tag="apch", bufs=1)
                nc.vector.tensor_scalar(out=apch[:, :w], in0=app[:, :w],
                                        scalar1=eb1[:, l:l + 1], scalar2=None,
                                        op0=ALU.add)
                bpp = psn.tile([P, NODE_CHUNK], f32, tag="psn", space="PSUM")
                nc.tensor.matmul(out=bpp[:, :w], lhsT=eW1s[l][:],
                                 rhs=h2T[:, jsl], start=True, stop=True)
                bch = sb2.tile([P, NODE_CHUNK], f32, tag="bch", bufs=1)
                nc.scalar.activation(out=bch[:, :w], in_=bpp[:, :w], func=ACT.Copy)
                # transpose chunks to node-major and store to DRAM tables
                for k0 in range(0, w, P):
                    kw = min(P, w - k0)
                    for (src_t, dst_t) in ((apch, Ap_nm), (bch, Bsh_nm)):
                        trp = psm.tile([P, P], f32, tag="psm", space="PSUM")
                        nc.tensor.transpose(out=trp[:kw, :], in_=src_t[:, k0:k0 + kw],
                                            identity=ident[:])
                        trs = sb2.tile([P, P], f32, tag="trs")
                        nc.vector.tensor_copy(out=trs[:kw, :], in_=trp[:kw, :])
                        nc.sync.dma_start(out=dst_t[j0 + k0:j0 + k0 + kw, :],
                                          in_=trs[:kw, :])

            # ---------- allgather B ----------
            if "nocoll" in dbg:
                nc.gpsimd.dma_start(out=Bfull[0:npc, :], in_=Bsh_nm[:])
                for cc in range(1, cores):
                    nc.gpsimd.dma_start(out=Bfull[cc * npc:(cc + 1) * npc, :],
                                        in_=Bsh_nm[:])
            else:
                nc.gpsimd.collective_compute(
                    "AllGather", ALU.bypass,
                    replica_groups=[list(range(cores))],
                    ins=[Bsh_nm.opt()], outs=[Bfull.opt()])

            # ---------- edge phase ----------
            t0 = 0
            for b in range(nblk):
                tb = int(t_lo[b] + t_hi[b])
                tlo = int(t_lo[b])
                nb = min(P, npc - b * P)
                Sp = pss.tile([P, P], f32, tag="Sp", space="PSUM")
                for q0 in range(0, tb, 4):
                    qn = min(4, tb - q0)
                    gtA = sb2.tile([P, 4, P], f32, tag="gtA", bufs=3)
                    gtB = sb2.tile([P, 4, P], f32, tag="gtB", bufs=3)
                    if "nogather" in dbg or "nogatherA" in dbg:
                        nc.gpsimd.memset(gtA[:, :qn, :], 0.0)
                    else:
                        nc.gpsimd.dma_gather(
                            out_ap=gtA[:, :qn, :], in_ap=Ap_nm[:],
                            idxs_ap=idxA[:, (t0 + q0) * 8:(t0 + q0 + qn) * 8],
                            num_idxs=qn * P, num_idxs_reg=qn * P, elem_size=P)
                    if "nogather" in dbg or "nogatherB" in dbg:
                        nc.gpsimd.memset(gtB[:, :qn, :], 0.0)
                    else:
                        # split group at the lo/hi stream boundary
                        for (s0, s1, tab) in (
                                (q0, min(q0 + qn, tlo), Bfull[0:half, :]),
                                (max(q0, tlo), q0 + qn, Bfull[half:n_nodes, :])):
                            if s1 <= s0:
                                continue
                            nc.gpsimd.dma_gather(
                                out_ap=gtB[:, s0 - q0:s1 - q0, :], in_ap=tab,
                                idxs_ap=idxB[:, s0 * 8 + t0 * 8:s1 * 8 + t0 * 8],
                                num_idxs=(s1 - s0) * P, num_idxs_reg=(s1 - s0) * P,
                                elem_size=P)
                    u = sb2.tile([P, 4, P], f32, tag="u", bufs=2)
                    nc.vector.tensor_tensor(out=u[:, :qn, :], in0=gtA[:, :qn, :],
                                            in1=gtB[:, :qn, :], op=ALU.add)
                    for t in range(q0, q0 + qn):
                        tt = t0 + t
                        v = sb3.tile([P, P], f32, tag="v")
                        nc.vector.scalar_tensor_tensor(
                            out=v[:], in0=wbc[l][:], scalar=eac[:, tt:tt + 1],
                            in1=u[:, t - q0, :], op0=ALU.mult, op1=ALU.add)
                        msg = sb3.tile([P, P], f32, tag="msg")
                        nc.scalar.activation(out=msg[:], in_=v[:], func=ACT.Relu)
                        sel = sb3.tile([P, P], f32, tag="sel")
                        nc.vector.tensor_scalar(
                            out=sel[:], in0=iota_f[:], scalar1=slot[:, tt:tt + 1],
                            scalar2=None, op0=ALU.is_equal)
                        nc.tensor.matmul(out=Sp[:], lhsT=msg[:], rhs=sel[:],
                                         start=(t == 0), stop=(t == tb - 1))
                t0 += tb
                # aggr^T = eW2^T @ S^T + eb2 (x) deg ; y = aggr^T + h2T
                Ss = sb2.tile([P, P], f32, tag="Ss")
                nc.vector.tensor_copy(out=Ss[:], in_=Sp[:])
                agp = psm.tile([P, P], f32, tag="psm", space="PSUM")
                nc.tensor.matmul(out=agp[:], lhsT=eW2[l][:], rhs=Ss[:],
                                 start=True, stop=False)
                nc.tensor.matmul(out=agp[:], lhsT=eb2row[0:1, l * P:(l + 1) * P],
                                 rhs=degrow[:, b * P:(b + 1) * P],
                                 start=False, stop=True)
                nsl = slice(b * P, b * P + nb)
                nc.vector.tensor_tensor(out=h2T[:, nsl], in0=agp[:, :nb],
                                        in1=h2T[:, nsl], op=ALU.add)

            # ---------- BN stats ----------
            sums = sb2.tile([P, n_chunks], f32, tag="sums")
            ssqs = sb2.tile([P, n_chunks], f32, tag="ssqs")
            for ci, (j0, w) in enumerate(chunks()):
                jsl = slice(j0, j0 + w)
                nc.vector.tensor_reduce(out=sums[:, ci:ci + 1], in_=h2T[:, jsl],
                                        axis=mybir.AxisListType.X, op=ALU.add)
                sq = sb2.tile([P, NODE_CHUNK], f32, tag="sq", bufs=1)
                nc.vector.tensor_tensor(out=sq[:, :w], in0=h2T[:, jsl],
                                        in1=h2T[:, jsl], op=ALU.mult)
                nc.vector.tensor_reduce(out=ssqs[:, ci:ci + 1], in_=sq[:, :w],
                                        axis=mybir.AxisListType.X, op=ALU.add)
            st = sb2.tile([P, 2], f32, tag="st")
            nc.vector.tensor_reduce(out=st[:, 0:1], in_=sums[:],
                                    axis=mybir.AxisListType.X, op=ALU.add)
            nc.vector.tensor_reduce(out=st[:, 1:2], in_=ssqs[:],
                                    axis=mybir.AxisListType.X, op=ALU.add)
            nc.gpsimd.dma_start(out=stats_in[:], in_=st[:])
            stg = sb2.tile([P, 2], f32, tag="stg")
            if "nocoll" in dbg:
                nc.sync.dma_start(out=stg[:], in_=stats_in[:])
            else:
                nc.gpsimd.collective_compute(
                    "AllReduce", ALU.add,
                    replica_groups=[list(range(cores))],
                    ins=[stats_in.opt()], outs=[stats_out.opt()])
                nc.sync.dma_start(out=stg[:], in_=stats_out[:])
            # mean/var -> scale/shift  (tiny [128,1] vector ops)
            mu = sb2.tile([P, 1], f32, tag="mu")
            nc.vector.tensor_scalar(out=mu[:], in0=stg[:, 0:1], scalar1=1.0 / n_nodes,
                                    scalar2=None, op0=ALU.mult)
            var = sb2.tile([P, 1], f32, tag="var")
            nc.vector.tensor_scalar(out=var[:], in0=stg[:, 1:2], scalar1=1.0 / n_nodes,
                                    scalar2=None, op0=ALU.mult)
            musq = sb2.tile([P, 1], f32, tag="musq")
            nc.vector.tensor_tensor(out=musq[:], in0=mu[:], in1=mu[:], op=ALU.mult)
            nc.vector.tensor_tensor(out=var[:], in0=var[:], in1=musq[:], op=ALU.subtract)
            std = sb2.tile([P, 1], f32, tag="std")
            nc.vector.tensor_scalar(out=std[:], in0=var[:], scalar1=BN_EPS,
                                    scalar2=None, op0=ALU.add)
            nc.scalar.sqrt(out=std[:], in_=std[:])
            rstd = sb2.tile([P, 1], f32, tag="rstd")
            nc.vector.reciprocal(out=rstd[:], in_=std[:])
            scal = sb2.tile([P, 1], f32, tag="scal")
            nc.vector.tensor_tensor(out=scal[:], in0=bng[:, l:l + 1], in1=rstd[:],
                                    op=ALU.mult)
            shft = sb2.tile([P, 1], f32, tag="shft")
            nc.vector.tensor_tensor(out=shft[:], in0=mu[:], in1=scal[:], op=ALU.mult)
            nc.vector.tensor_tensor(out=shft[:], in0=bnb[:, l:l + 1], in1=shft[:],
                                    op=ALU.subtract)
            # ---------- BN apply + relu (+ residual) ----------
            x_next = xn[l % 2]
            for j0, w in chunks():
                jsl = slice(j0, j0 + w)
                nc.vector.tensor_scalar(out=h2T[:, jsl], in0=h2T[:, jsl],
                                        scalar1=scal[:, 0:1], scalar2=shft[:, 0:1],
                                        op0=ALU.mult, op1=ALU.add)
                if l == 0:
                    nc.scalar.activation(out=x_next[:, jsl], in_=h2T[:, jsl],
                                         func=ACT.Relu)
                else:
                    nc.scalar.activation(out=h2T[:, jsl], in_=h2T[:, jsl],
                                         func=ACT.Relu)
                    nc.vector.tensor_tensor(out=x_next[:, jsl], in0=h2T[:, jsl],
                                            in1=x_cur[:, jsl], op=ALU.add)
            x_cur = x_next

        # ---------- global mean pool + final MLP ----------
        pp = psp.tile([P, n_graphs], f32, tag="pp", space="PSUM")
        for b in range(nblk):
            nb = min(P, npc - b * P)
            trp = psm.tile([P, P], f32, tag="psm", space="PSUM")
            nc.tensor.transpose(out=trp[:nb, :], in_=x_cur[:, b * P:b * P + nb],
                                identity=ident[:])
            ynm = sb2.tile([P, P], f32, tag="ynm")
            if nb < P:
                nc.vector.memset(ynm[:], 0.0)
            nc.vector.tensor_copy(out=ynm[:nb, :], in_=trp[:nb, :])
            nc.tensor.matmul(out=pp[:], lhsT=ynm[:],
                             rhs=poolsel[:, b * n_graphs:(b + 1) * n_graphs],
                             start=(b == 0), stop=(b == nblk - 1))
        pps = sb2.tile([P, n_graphs], f32, tag="pps")
        nc.vector.tensor_copy(out=pps[:], in_=pp[:])
        nc.gpsimd.dma_start(out=pool_in[:], in_=pps[:])
        pg = sb2.tile([P, n_graphs], f32, tag="pg")
        if "nocoll" in dbg:
            nc.sync.dma_start(out=pg[:], in_=pool_in[:])
        else:
            nc.gpsimd.collective_compute(
                "AllReduce", ALU.add,
                replica_groups=[list(range(cores))],
                ins=[pool_in.opt()], outs=[pool_out.opt()])
            nc.sync.dma_start(out=pg[:], in_=pool_out[:])
        nc.vector.tensor_tensor(out=pg[:], in0=pg[:], in1=invcnt[:], op=ALU.mult)
        z_ps = psm.tile([n_graphs, P], f32, tag="psm", space="PSUM")
        nc.tensor.matmul(out=z_ps[:], lhsT=pg[:], rhs=fc1W[:], start=True, stop=True)
        z = sb2.tile([n_graphs, P], f32, tag="z")
        nc.vector.tensor_tensor(out=z[:], in0=z_ps[:], in1=fc1b[:], op=ALU.add)
        nc.vector.tensor_scalar(out=z[:], in0=z[:], scalar1=0.0, scalar2=None,
                                op0=ALU.max)
        zt_ps = psm.tile([P, n_graphs], f32, tag="psm", space="PSUM")
        nc.tensor.transpose(out=zt_ps[:, :], in_=z[:], identity=ident[:n_graphs, :n_graphs])
        zt = sb2.tile([P, n_graphs], f32, tag="zt")
        nc.vector.tensor_copy(out=zt[:], in_=zt_ps[:])
        o_ps = psm.tile([n_graphs, 1], f32, tag="psm", space="PSUM")
        nc.tensor.matmul(out=o_ps[:], lhsT=zt[:], rhs=fc2W[:], start=True, stop=True)
        o_sb = sb2.tile([n_graphs, 1], f32, tag="osb")
        nc.vector.tensor_tensor(out=o_sb[:], in0=o_ps[:], in1=fc2b[:], op=ALU.add)
        nc.sync.dma_start(out=out_o[:], in_=o_sb[:])

    nc.finalize()
    return nc


# --------------------------------------------------------------------------
# host wrapper
# --------------------------------------------------------------------------

def make_in_maps(inputs, per_core, struct, inv_cnt, n_nodes, n_graphs, cores):
    npc = struct["npc"]
    x = np.asarray(inputs["x"], dtype=np.float32)
    eW1 = np.asarray(inputs["eW1"], dtype=np.float32)

    shared = dict(
        nW1=np.asarray(inputs["nW1"], np.float32).reshape(3 * P, P),
        nb1=np.asarray(inputs["nb1"], np.float32).T.reshape(P, 3).copy(),
        nW2=np.asarray(inputs["nW2"], np.float32).reshape(3 * P, P),
        nb2=np.asarray(inputs["nb2"], np.float32).T.reshape(P, 3).copy(),
        eW1d=eW1[:, :P, :].reshape(3 * P, P).copy(),
        eW1s=eW1[:, P:2 * P, :].reshape(3 * P, P).copy(),
        eb1=np.asarray(inputs["eb1"], np.float32).T.reshape(P, 3).copy(),
        wbc=np.concatenate([np.tile(eW1[l, 2 * P:2 * P + 1, :], (P, 1))
                            for l in range(3)], axis=0),
        eW2=np.asarray(inputs["eW2"], np.float32).reshape(3 * P, P),
        eb2row=np.asarray(inputs["eb2"], np.float32).reshape(1, 3 * P).copy(),
        bng=np.asarray(inputs["bng"], np.float32).T.reshape(P, 3).copy(),
        bnb=np.asarray(inputs["bnb"], np.float32).T.reshape(P, 3).copy(),
        fc1W=np.asarray(inputs["fc1W"], np.float32),
        fc1b=np.tile(np.asarray(inputs["fc1b"], np.float32)[None, :], (n_graphs, 1)),
        fc2W=np.asarray(inputs["fc2W"], np.float32).reshape(P, 1),
        fc2b=np.tile(np.asarray(inputs["fc2b"], np.float32).reshape(1, 1),
                     (n_graphs, 1)),
        inv_cnt=np.tile(inv_cnt[None, :], (P, 1)),
        ident=np.eye(P, dtype=np.float32),
    )
    in_maps = []
    for c in range(cores):
        m = dict(shared)
        m["xT"] = x[c * npc:(c + 1) * npc].T.copy()
        pc = per_core[c]
        m["idxA"] = pc["idxA"]
        m["idxB"] = pc["idxB"]
        m["slot"] = pc["slot"]
        m["ea"] = pc["eacol"]
        m["deg_row"] = pc["deg_row"]
        m["pool_sel"] = pc["pool_sel"]
        in_maps.append(m)
    return in_maps


def run(inputs, n_nodes=N_NODES, n_graphs=N_GRAPHS, cores=CORES, cache={}):
    edge_index = np.asarray(inputs["edge_index"])
    batch = np.asarray(inputs["batch"])
    per_core, struct, inv_cnt = preprocess(
        edge_index, batch, inputs["edge_attr"], n_nodes, n_graphs, cores)
    key = (n_nodes, n_graphs, cores, struct["T"],
           tuple(struct["t_lo"]), tuple(struct["t_hi"]))
    if key not in cache:
        cache.clear()
        cache[key] = build_program(n_nodes, n_graphs, cores, struct)
    nc = cache[key]
    in_maps = make_in_maps(inputs, per_core, struct, inv_cnt,
                           n_nodes, n_graphs, cores)
    results = bass2jax.run_bass_via_pjrt(nc, in_maps, n_cores=cores)
    return results[0]["out"].astype(np.float32)


def kernel(**inputs) -> np.ndarray:
    return run(inputs)
